# revision 23
# baseline (speedup 1.0000x reference)
"""GAT message-passing network on 8 Trainium2 NeuronCores (Bass/Tile).

Self-contained: takes full inputs, shards internally, returns full output.

Strategy:
- Host: add self-loops, greedy-balance nodes into 160 blocks of 125 nodes
  (edge counts equalized), relabel nodes; core c owns blocks 20c..20c+19.
- Device per conv: node-sharded matmul Z = x @ Wext^T producing
  [4x(head_feats|1.0) | a_src | a_dst] rows, AllGather the bf16 table G,
  then per dst-block: dma_gather of source rows (2304B, SWDGE queues 0/1),
  dma_gather of a_dst rows (256B, queue 2), ee = exp(leaky(asrc+adst)),
  one-hot S from iota-compare, per-head matmul S_h^T @ [h|1] accumulating
  weighted feature sums + softmax denominators in PSUM.  Normalize,
  head-mean, PE-transpose, BatchNorm stats via per-op accumulators +
  AllReduce, ReLU folded into the next conv's input load.
"""
import os
import sys

sys.path.insert(0, "/opt/trn_rl_repo")

import numpy as np
import ml_dtypes

import concourse.bass as bass
from concourse import bacc
import concourse.mybir as mybir
import concourse.tile as tile
from concourse.bass_utils import run_bass_kernel_spmd
from concourse.masks import make_identity

F32 = mybir.dt.float32
BF16 = mybir.dt.bfloat16
I16 = mybir.dt.int16
ALU = mybir.AluOpType
ACTF = mybir.ActivationFunctionType

N = int(os.environ.get("GNN_N", "20000"))
INC = 1024
HID = 256
HEADS = 4
NCORES = 8
BN_NODES = 125
NBLK = N // BN_NODES  # blocks total, BN_NODES nodes each
BPC = NBLK // NCORES  # blocks per core
NSHARD = N // NCORES
NT = NSHARD // BN_NODES  # node tiles per core
CW = 500 if NSHARD % 500 == 0 else 250  # MLP node-chunk width
ROWW = 1152           # gather-table row width (bf16) = 2304B
NCONV = int(os.environ.get("GNN_NCONV", "3"))   # debug: how many convs to run
NO_AR = os.environ.get("GNN_NOAR", "") == "1"   # debug: skip BN AllReduce
NO_G2 = os.environ.get("GNN_NOG2", "") == "1"   # debug: skip a_dst gather
EDGE_LVL = int(os.environ.get("GNN_EDGE_LVL", "2"))  # 0=gathers 1=+dve 2=+matmuls
ONE_Q = os.environ.get("GNN_ONEQ", "") == "1"   # debug: all gathers on queue 0
NO_BN = os.environ.get("GNN_NOBN", "") == "1"   # debug: skip transposes/stats/prep
BN_LVL = int(os.environ.get("GNN_BN_LVL", "3"))  # 1=transpose 2=+stats 3=+prep
EPS_BN = 1e-5
NEG = 0.2

# consts tensor column map
C_BM1 = 0             # 4 cols: mlp1 bias per o-chunk
C_BM2 = 4             # 4 cols
C_B3 = 8              # 1 col
C_BIAS = {0: 9, 1: 11}
C_GAM = {0: 13, 1: 15}
C_BET = {0: 17, 1: 19}
NCONSTS = 21

LAST_EXEC_NS = None
LAST_RESULTS = None
_PROGRAM_CACHE = {}


# ---------------------------------------------------------------- host prep
def _partition_graph(edge_index):
    """Self-loops + balanced blocks. Returns perm (old ids in new order),
    tpad, and per-core packed edge arrays."""
    import heapq

    src = np.concatenate([edge_index[0], np.arange(N, dtype=np.int64)])
    dst = np.concatenate([edge_index[1], np.arange(N, dtype=np.int64)])
    deg = np.bincount(dst, minlength=N)

    order = np.argsort(-deg, kind="stable")
    heap = [(0, b) for b in range(NBLK)]
    heapq.heapify(heap)
    load = np.zeros(NBLK, np.int64)
    cnt = np.zeros(NBLK, np.int64)
    blk_of_old = np.zeros(N, np.int32)
    for n in order:
        while True:
            _, b = heapq.heappop(heap)
            if cnt[b] < BN_NODES:
                blk_of_old[n] = b
                load[b] += deg[n]
                cnt[b] += 1
                heapq.heappush(heap, (load[b], b))
                break
    assert (cnt == BN_NODES).all()

    perm = np.lexsort((np.arange(N), blk_of_old))  # old ids in new order
    new_of_old = np.empty(N, np.int64)
    new_of_old[perm] = np.arange(N)

    src_n = new_of_old[src]
    dst_n = new_of_old[dst]
    blk_e = dst_n // BN_NODES

    tpad = int(np.ceil(load.max() / 128.0))

    e_order = np.argsort(blk_e, kind="stable")
    src_s = src_n[e_order]
    dst_s = dst_n[e_order]
    blk_s = blk_e[e_order]
    starts = np.searchsorted(blk_s, np.arange(NBLK + 1))

    src_pad = np.zeros((NBLK, tpad * 128), np.int64)
    dst_pad = np.zeros((NBLK, tpad * 128), np.int64)
    loc_pad = np.full((NBLK, tpad * 128), -1.0, np.float32)
    for b in range(NBLK):
        s, e = starts[b], starts[b + 1]
        k = e - s
        src_pad[b, :k] = src_s[s:e]
        dst_pad[b, :k] = dst_s[s:e]
        loc_pad[b, :k] = (dst_s[s:e] - b * BN_NODES).astype(np.float32)

    def wrap16(flat):
        # dma_gather index j lives at [j % 16, j // 16]; replicate to 128 parts
        w = flat.reshape(-1, 16).T.astype(np.int16)
        return np.tile(w, (8, 1)).copy()

    cores = []
    for c in range(NCORES):
        bsl = slice(c * BPC, (c + 1) * BPC)
        cores.append(
            dict(
                src16=wrap16(src_pad[bsl].reshape(-1)),
                dst16=wrap16(dst_pad[bsl].reshape(-1)),
                # per-edge j -> [j % 128, j // 128]
                dstloc=loc_pad[bsl].reshape(-1, 128).T.copy(),
            )
        )
    return perm, tpad, cores


def _pack_cols(vec, nchunks):
    """[nchunks*128] -> [128, nchunks] with [p, k] = vec[k*128+p]."""
    return np.ascontiguousarray(
        np.asarray(vec, np.float32).reshape(nchunks, 128).T
    )


def _build_wext(Wc, att_src, att_dst):
    """W [4*cout, cin] -> WextT [cin, 1032] bf16, col order
    [4*256 head feats | a_src 4 | a_dst 4]."""
    cout = att_src.shape[1]
    asrc_rows = np.stack(
        [att_src[h] @ Wc[h * cout : (h + 1) * cout] for h in range(HEADS)]
    )
    adst_rows = np.stack(
        [att_dst[h] @ Wc[h * cout : (h + 1) * cout] for h in range(HEADS)]
    )
    wext = np.concatenate([Wc, asrc_rows, adst_rows], axis=0)  # [1032, cin]
    return np.ascontiguousarray(wext.T).astype(ml_dtypes.bfloat16)


# ---------------------------------------------------------------- device
def _emit_zsh(nc, zs, zshard, m, chunks, gate):
    """PSUM chunks [0:512],[512:1024],[1024:1032] -> interleaved bf16 row
    [4x(256 feats|1.0) | asrc | adst | pad] and DMA into zshard."""
    zt = zs.tile([BN_NODES, ROWW], BF16, tag="zt")
    ztv = zt[:, :1028].rearrange("p (g c) -> p g c", g=4)  # [125, 4, 257]
    nc.vector.memset(ztv[:, :, 256:257], 1.0)
    nc.vector.memset(zt[:, 1036:], 0.0)  # pad cols (never read numerically)
    for cki in range(3):
        ps = chunks[cki]
        if cki < 2:
            out_ap = ztv[:, 2 * cki : 2 * cki + 2, 0:256]  # two heads
        else:
            out_ap = zt[:, 1028:1036]  # asrc | adst
        if gate is not None:
            nc.vector.tensor_scalar_mul(out=out_ap, in0=ps[:], scalar1=gate)
        else:
            nc.vector.tensor_copy(out=out_ap, in_=ps[:])
    nc.sync.dma_start(out=zshard[m * BN_NODES : (m + 1) * BN_NODES, :], in_=zt[:])


def _z_matmul(nc, zs, zps, zshard, lhs_tiles, rhs_w, nk, gate_sb):
    """Z = lhsT.T @ rhs (accumulate nk K-chunks); lhs_tiles(k, msl) -> AP."""
    for m in range(NT):
        msl = slice(m * BN_NODES, (m + 1) * BN_NODES)
        chunks = []
        for lo, wdt in ((0, 512), (512, 512), (1024, 8)):
            ps = zps.tile([BN_NODES, wdt], F32, space="PSUM", tag=f"zp{lo}")
            for k in range(nk):
                nc.tensor.matmul(
                    out=ps[:],
                    lhsT=lhs_tiles(k, msl),
                    rhs=rhs_w(k, lo, wdt),
                    start=(k == 0),
                    stop=(k == nk - 1),
                )
            chunks.append(ps)
        gate = gate_sb[:BN_NODES, m : m + 1] if gate_sb is not None else None
        _emit_zsh(nc, zs, zshard, m, chunks, gate)


def _build_program(tpad):
    ni = BPC * tpad * 128
    nc = bacc.Bacc(
        "TRN2",
        target_bir_lowering=False,
        debug=False,
        num_devices=NCORES,
        num_swdge_queues=3,
    )
    xT = nc.declare_dram_parameter("xT", [INC, NSHARD], F32, isOutput=False)
    src16 = nc.declare_dram_parameter("src16", [128, ni // 16], I16, isOutput=False)
    dst16 = nc.declare_dram_parameter("dst16", [128, ni // 16], I16, isOutput=False)
    dstloc = nc.declare_dram_parameter("dstloc", [128, ni // 128], F32, isOutput=False)
    negq = nc.declare_dram_parameter("negq", [128, 8], F32, isOutput=False)
    iotain = nc.declare_dram_parameter("iotain", [128, 128], F32, isOutput=False)
    w1T = nc.declare_dram_parameter("w1T", [INC, 1032], BF16, isOutput=False)
    w2T = nc.declare_dram_parameter("w2T", [HID, 1032], BF16, isOutput=False)
    w3T = nc.declare_dram_parameter("w3T", [HID, 1032], BF16, isOutput=False)
    wm1T = nc.declare_dram_parameter("wm1T", [INC, 512], BF16, isOutput=False)
    wm2T = nc.declare_dram_parameter("wm2T", [512, 512], BF16, isOutput=False)
    wm3T = nc.declare_dram_parameter("wm3T", [512, 1], BF16, isOutput=False)
    consts = nc.declare_dram_parameter("consts", [128, NCONSTS], F32, isOutput=False)
    outshard = nc.declare_dram_parameter("outshard", [NSHARD, HID], F32, isOutput=True)

    rg = [list(range(NCORES))]

    with tile.TileContext(nc) as tc:
        with tc.tile_pool(name="persist", bufs=1) as pp, \
             tc.tile_pool(name="dram", bufs=1, space="DRAM") as dp:
            # ---- persistent loads
            idx_src = pp.tile([128, ni // 16], I16)
            idx_dst = pp.tile([128, ni // 16], I16)
            loc_sb = pp.tile([128, ni // 128], F32)
            negq_sb = pp.tile([128, 8], F32)
            iota_sb = pp.tile([128, 128], F32)
            cst = pp.tile([128, NCONSTS], F32)
            ident = pp.tile([128, 128], F32)
            nc.sync.dma_start(out=idx_src[:], in_=src16[:])
            nc.sync.dma_start(out=idx_dst[:], in_=dst16[:])
            nc.sync.dma_start(out=loc_sb[:], in_=dstloc[:])
            nc.sync.dma_start(out=negq_sb[:], in_=negq[:])
            nc.sync.dma_start(out=iota_sb[:], in_=iotain[:])
            nc.sync.dma_start(out=cst[:], in_=consts[:])
            make_identity(nc, ident[:])

            w1_sb = pp.tile([128, 8, 1032], BF16)
            for k in range(8):
                nc.sync.dma_start(out=w1_sb[:, k, :], in_=w1T[k * 128 : (k + 1) * 128, :])
            w23_sb = pp.tile([128, 2, 2, 1032], BF16)  # [.., conv-1, kc, :]
            for ci, wt in ((0, w2T), (1, w3T)):
                for k in range(2):
                    nc.sync.dma_start(
                        out=w23_sb[:, ci, k, :], in_=wt[k * 128 : (k + 1) * 128, :]
                    )
            w_sb = pp.tile([128, NT], F32)       # gating weights per node tile
            xn_bf = pp.tile([128, 2, NSHARD], BF16)  # conv 1/2 input

            # DRAM intermediates
            zshard = dp.tile([NSHARD, ROWW], BF16)
            gtabs = [
                dp.tile(
                    [N, ROWW], BF16, addr_space="Shared",
                    name=f"gtab{i}", tag=f"gtab{i}",
                )
                for i in range(3)
            ]
            x2raw = dp.tile([HID, NSHARD], F32)
            st_in = dp.tile([128, 4], F32)
            st_outs = [
                dp.tile(
                    [128, 4], F32, addr_space="Shared",
                    name=f"stout{i}", tag=f"stout{i}",
                )
                for i in range(2)
            ]

            # ---- phase 1: x load/cast + gating MLP (node-chunked) + conv0 Z
            with tc.tile_pool(name="mlp", bufs=1) as mp:
                x_bf = mp.tile([128, 8, NSHARD], BF16)
                wm1_sb = mp.tile([128, 8, 512], BF16)
                nc.sync.dma_start(
                    out=wm1_sb[:], in_=wm1T[:].rearrange("(k p) o -> p k o", p=128)
                )
                wm2_sb = mp.tile([128, 4, 512], BF16)
                nc.sync.dma_start(
                    out=wm2_sb[:], in_=wm2T[:].rearrange("(k p) o -> p k o", p=128)
                )
                wm3_sb = mp.tile([128, 4, 1], BF16)
                nc.sync.dma_start(
                    out=wm3_sb[:], in_=wm3T[:].rearrange("(k p) o -> p k o", p=128)
                )
                NCH = NSHARD // CW  # node chunks for the MLP
                ms_ctx = tc.tile_pool(name="mlps", bufs=2)
                mps_ctx = tc.tile_pool(name="mpsum", bufs=4, space="PSUM")
                ms = ms_ctx.__enter__()
                mps = mps_ctx.__enter__()
                for m in range(NCH):  # noqa: E501
                    csl = slice(m * CW, (m + 1) * CW)
                    xd = ms.tile([128, 8, CW], BF16, tag="xd")
                    for k in range(8):
                        xf = ms.tile([128, CW], F32, tag="xf")
                        nc.sync.dma_start(out=xf[:], in_=xT[k * 128 : (k + 1) * 128, csl])
                        nc.vector.tensor_copy(out=x_bf[:, k, csl], in_=xf[:])
                        nc.scalar.activation(
                            out=xd[:, k, :], in_=xf[:], func=ACTF.Abs,
                            bias=negq_sb[:, k : k + 1],
                        )
                    h1 = ms.tile([128, 4, CW], BF16, tag="h1")
                    for oc in range(4):
                        ps = mps.tile([128, CW], F32, space="PSUM", tag="mp")
                        for k in range(8):
                            nc.tensor.matmul(
                                out=ps[:],
                                lhsT=wm1_sb[:, k, oc * 128 : (oc + 1) * 128],
                                rhs=xd[:, k, :],
                                start=(k == 0), stop=(k == 7),
                            )
                        nc.scalar.activation(
                            out=h1[:, oc, :], in_=ps[:], func=ACTF.Relu,
                            bias=cst[:, C_BM1 + oc : C_BM1 + oc + 1],
                        )
                    h2 = ms.tile([128, 4, CW], BF16, tag="h2")
                    for oc in range(4):
                        ps = mps.tile([128, CW], F32, space="PSUM", tag="mp")
                        for k in range(4):
                            nc.tensor.matmul(
                                out=ps[:],
                                lhsT=wm2_sb[:, k, oc * 128 : (oc + 1) * 128],
                                rhs=h1[:, k, :],
                                start=(k == 0), stop=(k == 3),
                            )
                        nc.scalar.activation(
                            out=h2[:, oc, :], in_=ps[:], func=ACTF.Relu,
                            bias=cst[:, C_BM2 + oc : C_BM2 + oc + 1],
                        )
                    for j in range(CW // BN_NODES):  # 4 node tiles of 125
                        ps = mps.tile([BN_NODES, 1], F32, space="PSUM", tag="wp")
                        for k in range(4):
                            nc.tensor.matmul(
                                out=ps[:],
                                lhsT=h2[:, k, j * BN_NODES : (j + 1) * BN_NODES],
                                rhs=wm3_sb[:, k, :],
                                start=(k == 0), stop=(k == 3),
                            )
                        mt = m * (CW // BN_NODES) + j
                        nc.scalar.activation(
                            out=w_sb[:BN_NODES, mt : mt + 1], in_=ps[:],
                            func=ACTF.Sigmoid, bias=cst[:BN_NODES, C_B3 : C_B3 + 1],
                        )

                mps_ctx.__exit__(None, None, None)

                # ---- conv0 Z matmul (x_bf still alive; keep mlps SBUF pool
                # open so z0s gets a fresh region - pool-region reuse under a
                # still-open outer pool raced in sim)
                with tc.tile_pool(name="z0s", bufs=3) as zs, \
                     tc.tile_pool(name="z0p", bufs=2, space="PSUM") as zps:
                    _z_matmul(
                        nc, zs, zps, zshard,
                        lambda k, msl: x_bf[:, k, msl],
                        lambda k, lo, wdt: w1_sb[:, k, lo : lo + wdt],
                        8, w_sb,
                    )
                ms_ctx.__exit__(None, None, None)

            for ci in range(NCONV):
                if ci > 0:
                    with tc.tile_pool(name=f"z{ci}s", bufs=3) as zs, \
                         tc.tile_pool(name=f"z{ci}p", bufs=2, space="PSUM") as zps:
                        _z_matmul(
                            nc, zs, zps, zshard,
                            lambda k, msl: xn_bf[:, k, msl],
                            lambda k, lo, wdt: w23_sb[:, ci - 1, k, lo : lo + wdt],
                            2, None,
                        )
                gtab = gtabs[ci]
                nc.gpsimd.collective_compute(
                    "AllGather", ALU.bypass, replica_groups=rg,
                    ins=[zshard[:].opt()], outs=[gtab[:].opt()],
                )
                # ---- edge phase
                with tc.tile_pool(name=f"e{ci}", bufs=2) as ep, \
                     tc.tile_pool(name=f"es{ci}", bufs=2) as es, \
                     tc.tile_pool(name=f"eh{ci}", bufs=1, space="PSUM") as hps, \
                     tc.tile_pool(name=f"et{ci}", bufs=2, space="PSUM") as tps:
                    sumacc = es.tile([128, 2, BPC], F32, tag="sumacc", bufs=1)
                    sqacc = es.tile([128, 2, BPC], F32, tag="sqacc", bufs=1)
                    pshead = [
                        hps.tile(
                            [BN_NODES, 257], F32, space="PSUM",
                            tag=f"ph{h}", name=f"ph{ci}_{h}",
                        )
                        for h in range(HEADS)
                    ]
                    for b in range(BPC):
                        isl = slice(b * tpad * 8, (b + 1) * tpad * 8)
                        hs = ep.tile([128, tpad, ROWW], BF16, tag="hs")
                        nc.gpsimd.dma_gather(
                            out_ap=hs[:], in_ap=gtab[:],
                            idxs_ap=idx_src[:, isl],
                            num_idxs=tpad * 128, num_idxs_reg=tpad * 128,
                            elem_size=ROWW, single_packet=False,
                            queue_num=0 if ONE_Q else b % 2,
                        )
                        g2 = ep.tile([128, tpad, 128], BF16, tag="g2")
                        if NO_G2:
                            nc.vector.memset(g2[:], 0.0)
                        else:
                            nc.gpsimd.dma_gather(
                                out_ap=g2[:], in_ap=gtab[:, 1024:],
                                idxs_ap=idx_dst[:, isl],
                                num_idxs=tpad * 128, num_idxs_reg=tpad * 128,
                                elem_size=128, elem_step=ROWW,
                                single_packet=False, queue_num=2,
                            )
                        eb = es.tile([128, tpad, HEADS], F32, tag="eb")
                        if EDGE_LVL >= 1:
                         nc.vector.tensor_add(
                            out=eb[:], in0=hs[:, :, 1028:1032], in1=g2[:, :, 8:12]
                        )
                        el = es.tile([128, tpad, HEADS], F32, tag="el")
                        ee = es.tile([128, tpad, HEADS], F32, tag="ee")
                        sblk = es.tile([128, tpad * BN_NODES], BF16, tag="sblk")
                        if EDGE_LVL >= 1:
                         nc.vector.tensor_scalar_mul(out=el[:], in0=eb[:], scalar1=NEG)
                         nc.vector.tensor_max(out=el[:], in0=el[:], in1=eb[:])
                         nc.scalar.activation(out=ee[:], in_=el[:], func=ACTF.Exp)
                         nc.vector.tensor_tensor(
                            out=sblk[:],
                            in0=loc_sb[:, b * tpad : (b + 1) * tpad].to_broadcast(
                                [128, tpad, BN_NODES]
                            ),
                            in1=iota_sb[:, :BN_NODES]
                            .rearrange("p (o n) -> p o n", o=1)
                            .to_broadcast([128, tpad, BN_NODES]),
                            op=ALU.is_equal,
                        )
                        sbig = es.tile([128, tpad, HEADS, BN_NODES], BF16, tag="sbig")
                        if EDGE_LVL >= 1:
                         nc.vector.tensor_tensor(
                            out=sbig[:],
                            in0=sblk[:]
                            .rearrange("p (t o n) -> p t o n", t=tpad, o=1)
                            .to_broadcast([128, tpad, HEADS, BN_NODES]),
                            in1=ee[:].to_broadcast([128, tpad, HEADS, BN_NODES]),
                            op=ALU.mult,
                        )
                        for t in range(tpad if EDGE_LVL >= 2 else 0):
                            for h in range(HEADS):
                                nc.tensor.matmul(
                                    out=pshead[h][:],
                                    lhsT=sbig[:, t, h, :],
                                    rhs=hs[:, t, h * 257 : (h + 1) * 257],
                                    start=(t == 0), stop=(t == tpad - 1),
                                )
                        # ---- block post: normalize + head mean
                        oacc = es.tile([BN_NODES, HID], F32, tag="oacc")
                        otmp = es.tile([BN_NODES, HID], F32, tag="otmp")
                        if EDGE_LVL < 2:
                            nc.vector.memset(oacc[:], 0.0)
                        for h in range(HEADS if EDGE_LVL >= 2 else 0):
                            d4 = es.tile([BN_NODES, 1], F32, tag=f"d4{h}")
                            nc.vector.tensor_scalar(
                                out=d4[:], in0=pshead[h][:, 256:257],
                                scalar1=4.0, scalar2=4e-16,
                                op0=ALU.mult, op1=ALU.add,
                            )
                            rec = es.tile([BN_NODES, 1], F32, tag=f"rec{h}")
                            nc.vector.reciprocal(out=rec[:], in_=d4[:])
                            tgt = oacc if h == 0 else otmp
                            nc.vector.tensor_scalar_mul(
                                out=tgt[:], in0=pshead[h][:, :HID], scalar1=rec[:]
                            )
                            if h > 0:
                                nc.vector.tensor_add(out=oacc[:], in0=oacc[:], in1=otmp[:])
                        if ci == NCONV - 1:
                            nc.sync.dma_start(
                                out=outshard[b * BN_NODES : (b + 1) * BN_NODES, :],
                                in_=oacc[:],
                            )
                        if ci < 2 and not NO_BN:
                            for ch in range(2):
                                pt = tps.tile(
                                    [128, BN_NODES], F32, space="PSUM", tag="pt"
                                )
                                nc.tensor.transpose(
                                    out=pt[:],
                                    in_=oacc[:, ch * 128 : (ch + 1) * 128],
                                    identity=ident[:BN_NODES, :BN_NODES],
                                )
                                xr = es.tile([128, BN_NODES], F32, tag="xr")
                                cb = C_BIAS[ci] + ch
                                nc.vector.tensor_scalar_add(
                                    out=xr[:], in0=pt[:],
                                    scalar1=cst[:, cb : cb + 1],
                                )
                                if BN_LVL >= 2:
                                    nc.vector.tensor_reduce(
                                        out=sumacc[:, ch, b : b + 1], in_=xr[:],
                                        axis=mybir.AxisListType.X, op=ALU.add,
                                    )
                                    sq = es.tile([128, BN_NODES], F32, tag="sq")
                                    nc.vector.tensor_mul(out=sq[:], in0=xr[:], in1=xr[:])
                                    nc.vector.tensor_reduce(
                                        out=sqacc[:, ch, b : b + 1], in_=sq[:],
                                        axis=mybir.AxisListType.X, op=ALU.add,
                                    )
                                nc.sync.dma_start(
                                    out=x2raw[
                                        ch * 128 : (ch + 1) * 128,
                                        b * BN_NODES : (b + 1) * BN_NODES,
                                    ],
                                    in_=xr[:],
                                )
                    if ci < 2 and not NO_BN and BN_LVL >= 3:
                        # ---- BN: allreduce stats, compute scale/shift, prep input
                        stat = es.tile([128, 4], F32, tag="stat", bufs=1)
                        nc.vector.reduce_sum(
                            out=stat[:, 0:2], in_=sumacc[:], axis=mybir.AxisListType.X
                        )
                        nc.vector.reduce_sum(
                            out=stat[:, 2:4], in_=sqacc[:], axis=mybir.AxisListType.X
                        )
                        nc.sync.dma_start(out=st_in[:], in_=stat[:])
                        st_out = st_outs[ci]
                        if not NO_AR:
                            nc.gpsimd.collective_compute(
                                "AllReduce", ALU.add, replica_groups=rg,
                                ins=[st_in[:].opt()], outs=[st_out[:].opt()],
                            )
                        gst = es.tile([128, 4], F32, tag="gst", bufs=1)
                        if NO_AR:
                            nc.sync.dma_start(out=gst[:], in_=st_in[:])
                        else:
                            nc.sync.dma_start(out=gst[:], in_=st_out[:])
                        scale = es.tile([128, 2], F32, tag="scale", bufs=1)
                        shift = es.tile([128, 2], F32, tag="shift", bufs=1)
                        mu = es.tile([128, 2], F32, tag="mu", bufs=1)
                        var = es.tile([128, 2], F32, tag="var", bufs=1)
                        sd = es.tile([128, 2], F32, tag="sd", bufs=1)
                        nc.vector.tensor_scalar_mul(out=mu[:], in0=gst[:, 0:2], scalar1=1.0 / N)
                        nc.vector.tensor_scalar_mul(out=var[:], in0=gst[:, 2:4], scalar1=1.0 / N)
                        nc.vector.tensor_mul(out=sd[:], in0=mu[:], in1=mu[:])
                        nc.vector.tensor_sub(out=var[:], in0=var[:], in1=sd[:])
                        nc.vector.tensor_scalar_add(out=var[:], in0=var[:], scalar1=EPS_BN)
                        nc.scalar.activation(out=sd[:], in_=var[:], func=ACTF.Sqrt)
                        nc.vector.reciprocal(out=scale[:], in_=sd[:])
                        gcol = C_GAM[ci]
                        nc.vector.tensor_tensor(
                            out=scale[:], in0=scale[:], in1=cst[:, gcol : gcol + 2],
                            op=ALU.mult,
                        )
                        nc.vector.tensor_mul(out=shift[:], in0=mu[:], in1=scale[:])
                        bcol = C_BET[ci]
                        nc.vector.tensor_sub(
                            out=shift[:], in0=cst[:, bcol : bcol + 2], in1=shift[:]
                        )
                        for ch in range(2):
                            for s in range(NSHARD // CW):
                                ssl = slice(s * CW, (s + 1) * CW)
                                xrt = es.tile([128, CW], F32, tag="xrt")
                                nc.sync.dma_start(
                                    out=xrt[:],
                                    in_=x2raw[ch * 128 : (ch + 1) * 128, ssl],
                                )
                                xsc = es.tile([128, CW], F32, tag="xsc")
                                nc.vector.tensor_scalar(
                                    out=xsc[:], in0=xrt[:],
                                    scalar1=scale[:, ch : ch + 1],
                                    scalar2=shift[:, ch : ch + 1],
                                    op0=ALU.mult, op1=ALU.add,
                                )
                                nc.scalar.activation(
                                    out=xn_bf[:, ch, ssl], in_=xsc[:], func=ACTF.Relu
                                )
    nc.finalize()
    return nc


# ---------------------------------------------------------------- kernel
def kernel(x, edge_index, question_node, edge_attr, question_edge, params):
    global LAST_EXEC_NS, LAST_RESULTS
    x = np.asarray(x, np.float32)
    edge_index = np.asarray(edge_index)
    q = np.asarray(question_node, np.float32).reshape(-1)

    perm, tpad, cores = _partition_graph(edge_index.astype(np.int64))

    convs = params["convs"]
    bns = params["bns"]
    mlp = params["mlp_x"]

    wT = [
        _build_wext(
            np.asarray(c["W"], np.float32),
            np.asarray(c["att_src"], np.float32),
            np.asarray(c["att_dst"], np.float32),
        )
        for c in convs
    ]
    wm = [
        np.ascontiguousarray(np.asarray(l["W"], np.float32).T).astype(
            ml_dtypes.bfloat16
        )
        for l in mlp
    ]

    consts = np.zeros((128, NCONSTS), np.float32)
    consts[:, C_BM1 : C_BM1 + 4] = _pack_cols(mlp[0]["b"], 4)
    consts[:, C_BM2 : C_BM2 + 4] = _pack_cols(mlp[1]["b"], 4)
    consts[:, C_B3] = float(np.asarray(mlp[2]["b"], np.float32)[0])
    for ci in (0, 1):
        consts[:, C_BIAS[ci] : C_BIAS[ci] + 2] = _pack_cols(convs[ci]["bias"], 2)
        consts[:, C_GAM[ci] : C_GAM[ci] + 2] = _pack_cols(bns[ci]["gamma"], 2)
        consts[:, C_BET[ci] : C_BET[ci] + 2] = _pack_cols(bns[ci]["beta"], 2)

    negq = -_pack_cols(q, 8)
    iota = np.tile(np.arange(128, dtype=np.float32), (128, 1))

    if tpad not in _PROGRAM_CACHE:
        _PROGRAM_CACHE[tpad] = _build_program(tpad)
    nc = _PROGRAM_CACHE[tpad]

    xp = x[perm]  # new-id order
    in_maps = []
    for c in range(NCORES):
        xs = xp[c * NSHARD : (c + 1) * NSHARD]
        in_maps.append(
            dict(
                xT=np.ascontiguousarray(xs.T),
                src16=cores[c]["src16"],
                dst16=cores[c]["dst16"],
                dstloc=cores[c]["dstloc"],
                negq=negq,
                iotain=iota,
                w1T=wT[0], w2T=wT[1], w3T=wT[2],
                wm1T=wm[0], wm2T=wm[1], wm3T=wm[2],
                consts=consts,
            )
        )
    trace = os.environ.get("GNN_TRACE", "") == "1"
    res = run_bass_kernel_spmd(nc, in_maps, core_ids=list(range(NCORES)), trace=trace)
    LAST_EXEC_NS = res.exec_time_ns
    LAST_RESULTS = res

    out_new = np.concatenate(
        [res.results[c]["outshard"] for c in range(NCORES)], axis=0
    )
    out = np.empty((N, HID), np.float32)
    out[perm] = out_new
    out = out + np.asarray(convs[2]["bias"], np.float32)[None, :]
    return (out, np.asarray(edge_attr))
